# revision 2
# baseline (speedup 1.0000x reference)
"""DeepseekV3 MLA decode attention kernel for 8 Trainium2 NeuronCores, v2.

Sharding: 4 head-groups (32 heads) x 2 batch-groups (8 batches); core =
(hg, bg). Weights sharded by head, KV cache by batch.

v2 restructure vs v1 (informed by HW microbenchmarks: normal-mode ldweights
is fast ~18ns even for 128x128; DoubleRow halves streaming but loads weights
at 1 col/cycle; DMA ~350-420 GB/s):
  - q_nope projection computed directly transposed (wq tiles as PE weights,
    tiny B-wide streams) -> qnT with no transposes.
  - q_pe projection via DoubleRow fp8 (stream-heavy form), rope on DVE,
    then 32 small PE transposes.
  - scores computed TRANSPOSED: pst[n-tile, h] = ckvT-tiles^T @ qlatT,
    whole batch accumulates in ONE psum bank [128,16,32]; softmax exp is a
    single [128,512] activation; no p transposes.
  - row sums via ones-matmul (partition reduction on PE), 1/sum broadcast
    via f32 outer-product matmul.
  - o computed TRANSPOSED: po[c, h] = ckv-tiles^T @ pT (bf16 for accuracy);
    no o transposes; oT feeds the output projection directly.
"""

import sys

for _p in ("/opt/trn_rl_repo", "/root/.axon_site/_ro/trn_rl_repo"):
    if _p not in sys.path:
        sys.path.append(_p)

import numpy as np
import ml_dtypes

import concourse.bass as bass
import concourse.bacc as bacc
import concourse.tile as tile
from concourse import mybir
from concourse.bass_utils import run_bass_kernel_spmd
from concourse.masks import make_identity

BF16 = mybir.dt.bfloat16
FP8 = mybir.dt.float8e4
F32 = mybir.dt.float32
NPBF = ml_dtypes.bfloat16
NPF8 = ml_dtypes.float8_e4m3
FP8S = 16.0

NUM_HEADS = 128
QK_NOPE = 128
QK_ROPE = 64
V_HEAD = 128
QD = 192
C = 512
L = 1536
SCALE = 1.0 / float(np.sqrt(192.0))

HG = 4
BGQ = 2
N_CORES = 8

_BUILD_CACHE = {}


def _build(n_cached, B, H, rep=1):
    NT = n_cached // 128       # 16 n-tiles
    LT = L // 128              # 12
    LP = LT // 2               # 6 l-pairs
    PE_COLS = H * QK_ROPE      # 2048
    assert n_cached % 128 == 0 and H == 32 and B == 8

    nc = bacc.Bacc("TRN2", target_bir_lowering=False, debug=False)

    qdnT_dr = nc.dram_tensor("qdnT_dr", [128, LP, 2, B], FP8, kind="ExternalInput")
    wq_nope = nc.dram_tensor("wq_nope", [4, 128, 8, LT, 128], FP8, kind="ExternalInput")
    wq_pe = nc.dram_tensor("wq_pe", [128, LP, 2, PE_COLS], FP8, kind="ExternalInput")
    w_ukvT = nc.dram_tensor("w_ukvT", [128, H, C], FP8, kind="ExternalInput")
    w_uvT = nc.dram_tensor("w_uvT", [4, 128, 8, 4, V_HEAD], BF16, kind="ExternalInput")
    ckvT_sc = nc.dram_tensor("ckvT_sc", [B, 128, 4, n_cached], FP8, kind="ExternalInput")
    ckv_nc = nc.dram_tensor("ckv_nc", [B, 128, NT, C], BF16, kind="ExternalInput")
    kpeT = nc.dram_tensor("kpeT", [B, QK_ROPE, n_cached], FP8, kind="ExternalInput")
    ckvnewT = nc.dram_tensor("ckvnewT", [128, 4, B], FP8, kind="ExternalInput")
    ckvnew_nc = nc.dram_tensor("ckvnew_nc", [1, B, C], BF16, kind="ExternalInput")
    knewT = nc.dram_tensor("knewT", [QK_ROPE, B], FP8, kind="ExternalInput")
    cos_rep = nc.dram_tensor("cos_rep", [B, H * 32], F32, kind="ExternalInput")
    sin_rep = nc.dram_tensor("sin_rep", [B, H * 32], F32, kind="ExternalInput")
    out = nc.dram_tensor("out", [B, H, V_HEAD], F32, kind="ExternalOutput")

    with tile.TileContext(nc) as tc:
        # Cache-streaming pools first so their SBUF never moves.
        with (
            tc.tile_pool(name="ckvT_p", bufs=3) as ckvT_p,
            tc.tile_pool(name="ckvnc_p", bufs=2) as ckvnc_p,
            tc.tile_pool(name="kpeT_p", bufs=3) as kpeT_p,
            tc.tile_pool(name="wuv_p", bufs=4) as wuv_p,
            tc.tile_pool(name="consts", bufs=1) as consts,
            tc.tile_pool(name="persist", bufs=1) as persist,
        ):
            for _r in range(rep):
                ident = consts.tile([128, 128], BF16, name=f"ident{_r}")
                make_identity(nc, ident)
                ones_col = consts.tile([128, 1], BF16, name=f"onec{_r}")
                nc.gpsimd.memset(ones_col, 1.0)
                ones_row = consts.tile([1, 128], F32, name=f"oner{_r}")
                nc.gpsimd.memset(ones_row, 1.0)
                qdn_sb = consts.tile([128, LP, 2, B], FP8, name=f"qdn{_r}")
                nc.sync.dma_start(out=qdn_sb, in_=qdnT_dr[:, :, :, :])
                cos_sb = consts.tile([B, H * 32], F32, name=f"cos{_r}")
                nc.sync.dma_start(out=cos_sb, in_=cos_rep[:, :])
                sin_sb = consts.tile([B, H * 32], F32, name=f"sin{_r}")
                nc.sync.dma_start(out=sin_sb, in_=sin_rep[:, :])
                knewT_sb = consts.tile([QK_ROPE, B], FP8, name=f"knew{_r}")
                nc.sync.dma_start(out=knewT_sb, in_=knewT[:, :])
                ckvnewT_sb = consts.tile([128, 4, B], FP8, name=f"cknT{_r}")
                nc.sync.dma_start(out=ckvnewT_sb, in_=ckvnewT[:, :, :])
                ckvnewnc_sb = consts.tile([1, B, C], BF16, name=f"cknN{_r}")
                nc.sync.dma_start(out=ckvnewnc_sb, in_=ckvnew_nc[:, :, :])

                qnT = persist.tile([128, H, B], FP8, name=f"qnT{_r}")
                qpeT = persist.tile([QK_ROPE, H, B], FP8, name=f"qpeT{_r}")
                qlatT = persist.tile([128, 4, H, B], FP8, name=f"qlatT{_r}")
                oT_all = persist.tile([128, 4, H, B], BF16, name=f"oT{_r}")
                out_sb = persist.tile([B, H, V_HEAD], F32, name=f"outsb{_r}")

                # ---------------- Phase A ------------------------------------
                with (
                    tc.tile_pool(name="wqpe_p", bufs=1) as wqpe_p,
                    tc.tile_pool(name="wqn_p", bufs=2) as wqn_p,
                    tc.tile_pool(name="wukv_p", bufs=2) as wukv_p,
                    tc.tile_pool(name="s1a", bufs=1) as s1a,
                    tc.tile_pool(name="ps_q", bufs=2, space="PSUM") as ps_q,
                    tc.tile_pool(name="ps_n", bufs=2, space="PSUM") as ps_n,
                    tc.tile_pool(name="ps_t", bufs=2, space="PSUM") as ps_t,
                ):
                    # q_pe projection (DoubleRow, stream-heavy) + rope
                    wqpe = wqpe_p.tile([128, LP, 2, PE_COLS], FP8, tag="wqpe")
                    nc.sync.dma_start(out=wqpe, in_=wq_pe[:, :, :, :])
                    qpe_bf = s1a.tile([B, H, QK_ROPE], F32)
                    for ch in range(PE_COLS // 512):
                        psq = ps_q.tile([B, 512], F32, tag="psq")
                        for t in range(LT):
                            nc.tensor.matmul(
                                psq,
                                lhsT=qdn_sb[:, t // 2, t % 2, :],
                                rhs=wqpe[:, t // 2, t % 2, ch * 512:(ch + 1) * 512],
                                start=(t == 0), stop=(t == LT - 1),
                            )
                        nc.vector.tensor_scalar_mul(
                            qpe_bf.rearrange("b h d -> b (h d)")[:, ch * 512:(ch + 1) * 512],
                            psq, 1.0 / (FP8S * FP8S),
                        )
                    # rope (interleaved pairs -> half-split rotated)
                    xpairs = qpe_bf.rearrange("b h (i two) -> b h i two", two=2)
                    xe = xpairs[:, :, :, 0]
                    xo = xpairs[:, :, :, 1]
                    cos3 = cos_sb.rearrange("b (h i) -> b h i", i=32)
                    sin3 = sin_sb.rearrange("b (h i) -> b h i", i=32)
                    qpe_rot = s1a.tile([B, H, QK_ROPE], BF16)
                    tmp = s1a.tile([B, 2, H, 32], F32)
                    nc.vector.tensor_mul(tmp[:, 0], xe, cos3)
                    nc.vector.tensor_mul(tmp[:, 1], xo, sin3)
                    nc.vector.tensor_sub(qpe_rot[:, :, 0:32], tmp[:, 0], tmp[:, 1])
                    nc.vector.tensor_mul(tmp[:, 0], xo, cos3)
                    nc.vector.tensor_mul(tmp[:, 1], xe, sin3)
                    nc.vector.tensor_add(qpe_rot[:, :, 32:64], tmp[:, 0], tmp[:, 1])
                    # transpose q_pe per head -> qpeT fp8
                    TCH = 8
                    for hc in range(H // TCH):
                        ptp = ps_t.tile([QK_ROPE, TCH, B], BF16, tag="tr")
                        for hh in range(TCH):
                            h = hc * TCH + hh
                            nc.tensor.transpose(
                                ptp[:, hh, :], qpe_rot[:, h, :], ident[:B, :B]
                            )
                        nc.vector.tensor_scalar_mul(
                            qpeT[:, hc * TCH:(hc + 1) * TCH, :], ptp, FP8S
                        )

                    # q_nope projection (directly transposed) interleaved
                    # with absorption per 8-head group
                    for g in range(4):
                        wqn = wqn_p.tile([128, 8, LT, 128], FP8, tag="wqn")
                        nc.sync.dma_start(out=wqn, in_=wq_nope[g])
                        wukv = wukv_p.tile([128, 8, C], FP8, tag="wukv")
                        nc.sync.dma_start(
                            out=wukv, in_=w_ukvT[:, g * 8:(g + 1) * 8, :]
                        )
                        for k in range(8):
                            jj = g * 8 + k
                            psn = ps_n.tile([128, B], F32, tag="psn")
                            for u in range(LP):
                                nc.tensor.matmul(
                                    psn,
                                    lhsT=wqn[:, k, 2 * u:2 * u + 2, :],
                                    rhs=qdn_sb[:, u, :, :],
                                    start=(u == 0), stop=(u == LP - 1),
                                    perf_mode=mybir.MatmulPerfMode.DoubleRow,
                                )
                            nc.vector.tensor_scalar_mul(qnT[:, jj, :], psn, 1.0 / FP8S)
                        pa = ps_t.tile([128, 4, 8, B], F32, tag="abs")
                        for hh in range(8):
                            for ct in range(4):
                                nc.tensor.matmul(
                                    pa[:, ct, hh, :],
                                    lhsT=wukv[:, hh, ct * 128:(ct + 1) * 128],
                                    rhs=qnT[:, g * 8 + hh, :],
                                    start=True, stop=True,
                                )
                        nc.vector.tensor_scalar_mul(
                            qlatT[:, :, g * 8:(g + 1) * 8, :], pa, 1.0 / FP8S
                        )

                # ---------------- Phase B: attention per batch ----------------
                with (
                    tc.tile_pool(name="p_p", bufs=2) as p_p,
                    tc.tile_pool(name="sum_p", bufs=2) as sum_p,
                    tc.tile_pool(name="ps_s", bufs=2, space="PSUM") as ps_s,
                    tc.tile_pool(name="ps_o", bufs=2, space="PSUM") as ps_o,
                    tc.tile_pool(name="ps_m", bufs=1, space="PSUM") as ps_m,
                ):
                    wvts = []
                    for hc in range(4):
                        wvt = wuv_p.tile([128, 8, 4, V_HEAD], BF16, tag="wuv")
                        nc.sync.dma_start(out=wvt, in_=w_uvT[hc])
                        wvts.append(wvt)
                    for b in range(B):
                        ckvT_t = ckvT_p.tile([128, 4, n_cached], FP8, tag="ckvT")
                        nc.sync.dma_start(out=ckvT_t, in_=ckvT_sc[b])
                        ckvnc_t = ckvnc_p.tile([128, NT, C], BF16, tag="ckvnc")
                        nc.sync.dma_start(out=ckvnc_t, in_=ckv_nc[b])
                        kpeT_t = kpeT_p.tile([QK_ROPE, n_cached], FP8, tag="kpeT")
                        nc.sync.dma_start(out=kpeT_t, in_=kpeT[b])

                        # scoresT: whole batch in one psum bank [128, NT, 32]
                        pst = ps_s.tile([128, NT, H], F32, tag="st")
                        for nt in range(NT):
                            for cp in range(2):
                                nc.tensor.matmul(
                                    pst[:, nt, :],
                                    lhsT=ckvT_t[:, 2 * cp:2 * cp + 2, nt * 128:(nt + 1) * 128],
                                    rhs=qlatT[:, 2 * cp:2 * cp + 2, :, b],
                                    start=(cp == 0), stop=False,
                                    perf_mode=mybir.MatmulPerfMode.DoubleRow,
                                )
                            nc.tensor.matmul(
                                pst[:, nt, :],
                                lhsT=kpeT_t[:, nt * 128:(nt + 1) * 128],
                                rhs=qpeT[:, :, b],
                                start=False, stop=True,
                            )
                        pst_t = ps_m.tile([1, H], F32, tag="stt")
                        for ct in range(4):
                            nc.tensor.matmul(
                                pst_t,
                                lhsT=ckvnewT_sb[:, ct, b:b + 1],
                                rhs=qlatT[:, ct, :, b],
                                start=(ct == 0), stop=False,
                            )
                        nc.tensor.matmul(
                            pst_t, lhsT=knewT_sb[:, b:b + 1], rhs=qpeT[:, :, b],
                            start=False, stop=True,
                        )
                        # exp: one activation for the whole batch + tail
                        pT = p_p.tile([128, NT, H], BF16, tag="pT")
                        pT_t = p_p.tile([1, H], BF16, tag="pTt")
                        nc.scalar.activation(
                            pT, pst, mybir.ActivationFunctionType.Exp,
                            scale=SCALE / (FP8S * FP8S),
                        )
                        nc.scalar.activation(
                            pT_t, pst_t, mybir.ActivationFunctionType.Exp,
                            scale=SCALE / (FP8S * FP8S),
                        )
                        # sums over n (partition reduction via ones-matmul)
                        sums = ps_m.tile([1, H], F32, tag="sums")
                        for nt in range(NT):
                            nc.tensor.matmul(
                                sums, lhsT=ones_col, rhs=pT[:, nt, :],
                                start=(nt == 0), stop=False,
                            )
                        nc.tensor.matmul(
                            sums, lhsT=ones_col[0:1, :], rhs=pT_t,
                            start=False, stop=True,
                        )
                        rcpT = sum_p.tile([1, H], F32, tag="rcp")
                        nc.vector.reciprocal(rcpT, sums)
                        bc = ps_m.tile([128, H], F32, tag="bc")
                        nc.tensor.matmul(bc, lhsT=ones_row, rhs=rcpT,
                                         start=True, stop=True)
                        bc_sb = sum_p.tile([128, H], F32, tag="bcs")
                        nc.vector.tensor_copy(bc_sb, bc)

                        # oT = ckv_tiles^T @ pT  [c, h] per ct chunk
                        po = ps_o.tile([128, 4, H], F32, tag="po")
                        for ct in range(4):
                            for nt in range(NT):
                                nc.tensor.matmul(
                                    po[:, ct, :],
                                    lhsT=ckvnc_t[:, nt, ct * 128:(ct + 1) * 128],
                                    rhs=pT[:, nt, :],
                                    start=(nt == 0), stop=False,
                                )
                            nc.tensor.matmul(
                                po[:, ct, :],
                                lhsT=ckvnewnc_sb[0:1, b, ct * 128:(ct + 1) * 128],
                                rhs=pT_t,
                                start=False, stop=True,
                            )
                        for ct in range(4):
                            nc.vector.tensor_mul(
                                oT_all[:, ct, :, b], po[:, ct, :], bc_sb
                            )

                # ---------------- Phase C: output projection ------------------
                with (
                    tc.tile_pool(name="ps_r", bufs=2, space="PSUM") as ps_r,
                ):
                    for hc in range(4):
                        wvt = wvts[hc]
                        for hh in range(8):
                            h = hc * 8 + hh
                            psr = ps_r.tile([B, V_HEAD], F32, tag="r")
                            for ct in range(4):
                                nc.tensor.matmul(
                                    psr, lhsT=oT_all[:, ct, h, :],
                                    rhs=wvt[:, hh, ct, :],
                                    start=(ct == 0), stop=(ct == 3),
                                )
                            nc.vector.tensor_copy(out_sb[:, h, :], psr)
                        nc.sync.dma_start(
                            out=out[:, hc * 8:(hc + 1) * 8, :],
                            in_=out_sb[:, hc * 8:(hc + 1) * 8, :],
                        )

    nc.compile()
    return nc


def _get_build(n_cached, B, H, rep=1):
    key = (n_cached, B, H, rep)
    if key not in _BUILD_CACHE:
        _BUILD_CACHE[key] = _build(n_cached, B, H, rep)
    return _BUILD_CACHE[key]


def prepare_in_maps(**inputs):
    q = np.asarray(inputs["q_normed_dn"], dtype=np.float32)          # [16,1,1536]
    ckv_new = np.asarray(inputs["compressed_kv"], dtype=np.float32)  # [16,1,512]
    k_pe = np.asarray(inputs["k_pe"], dtype=np.float32)              # [16,1,1,64]
    pos = np.asarray(inputs["position_ids"]).astype(np.int64)        # [16,1]
    start_pos = int(inputs["start_pos"])
    ckv_cache = np.asarray(inputs["ckv_cache"], dtype=np.float32)
    kpe_cache = np.asarray(inputs["k_pe_cache"], dtype=np.float32)
    sin_c = np.asarray(inputs["sin_cache"], dtype=np.float32)
    cos_c = np.asarray(inputs["cos_cache"], dtype=np.float32)
    wkv_b = np.asarray(inputs["wkv_b"], dtype=np.float32)            # [128,256,512]
    wq_b = np.asarray(inputs["wq_b"], dtype=np.float32)              # [24576,1536]

    bsz = q.shape[0]
    B = bsz // BGQ
    H = NUM_HEADS // HG
    n_cached = start_pos
    NT = n_cached // 128
    LT = L // 128
    LP = LT // 2

    cos_g = cos_c[pos[:, 0]][:, :32]                                 # [16,32]
    sin_g = sin_c[pos[:, 0]][:, :32]
    cos_rep = np.tile(cos_g, (1, H)).astype(np.float32)              # [16,H*32]
    sin_rep = np.tile(sin_g, (1, H)).astype(np.float32)

    # host-side rope of the new-token k_pe (tiny)
    kp = k_pe[:, 0, 0, :]                                            # [16,64]
    kxe, kxo = kp[:, 0::2], kp[:, 1::2]
    k_roped = np.concatenate(
        [kxe * cos_g - kxo * sin_g, kxo * cos_g + kxe * sin_g], axis=1
    )                                                                # [16,64]

    wq_r = wq_b.reshape(NUM_HEADS, QD, L)

    wqn_shards, wqpe_shards, wukv_shards, wuv_shards = [], [], [], []
    for hg in range(HG):
        hs = slice(hg * H, (hg + 1) * H)
        nope = wq_r[hs, :QK_NOPE, :]                                 # [32,128,1536]
        # [p, h, t, d] -> chunks [4][128][8][12][128]
        a = nope.transpose(2, 0, 1).reshape(LT, 128, H, QK_NOPE)
        a = a.transpose(1, 2, 0, 3).reshape(128, 4, 8, LT, QK_NOPE)
        wqn_shards.append(
            np.ascontiguousarray(a.transpose(1, 0, 2, 3, 4) * FP8S).astype(NPF8)
        )
        pe = wq_r[hs, QK_NOPE:, :]                                   # [32,64,1536]
        b_ = pe.reshape(H * QK_ROPE, L).T                            # [1536, 2048]
        b_ = b_.reshape(LP, 2, 128, H * QK_ROPE).transpose(2, 0, 1, 3)
        wqpe_shards.append(np.ascontiguousarray(b_ * FP8S).astype(NPF8))
        wukv = wkv_b[hs, :QK_NOPE, :]                                # [32,128,512]
        wukv_shards.append(
            np.ascontiguousarray(wukv.transpose(1, 0, 2) * FP8S).astype(NPF8)
        )
        wuv = wkv_b[hs, QK_NOPE:, :]                                 # [32,128,512]
        c_ = wuv.transpose(2, 0, 1).reshape(4, 128, H, V_HEAD)
        c_ = c_.transpose(1, 2, 0, 3).reshape(128, 4, 8, 4, V_HEAD)
        wuv_shards.append(
            np.ascontiguousarray(c_.transpose(1, 0, 2, 3, 4)).astype(NPBF)
        )

    ckvT_shards, ckvnc_shards, kpeT_shards = [], [], []
    qT_shards, cknT_shards, cknN_shards, knew_shards = [], [], [], []
    cos_shards, sin_shards = [], []
    for bg in range(BGQ):
        bs = slice(bg * B, (bg + 1) * B)
        cache = ckv_cache[bs, :n_cached, :]                          # [8,2048,512]
        a = cache.reshape(B, n_cached, 4, 128).transpose(0, 3, 2, 1)
        ckvT_shards.append(np.ascontiguousarray(a * FP8S).astype(NPF8))
        b_ = cache.reshape(B, NT, 128, C).transpose(0, 2, 1, 3)
        ckvnc_shards.append(np.ascontiguousarray(b_).astype(NPBF))
        kp_ = kpe_cache[bs, :n_cached, :].transpose(0, 2, 1)
        kpeT_shards.append(np.ascontiguousarray(kp_ * FP8S).astype(NPF8))
        qt = q[bs, 0, :].T.reshape(LP, 2, 128, B).transpose(2, 0, 1, 3)
        qT_shards.append(np.ascontiguousarray(qt * FP8S).astype(NPF8))
        ckn = ckv_new[bs, 0, :]                                      # [8,512]
        cknT_shards.append(np.ascontiguousarray(
            ckn.reshape(B, 4, 128).transpose(2, 1, 0) * FP8S).astype(NPF8))
        cknN_shards.append(np.ascontiguousarray(ckn[None]).astype(NPBF))
        knew_shards.append(np.ascontiguousarray(
            k_roped[bs].T * FP8S).astype(NPF8))
        cos_shards.append(np.ascontiguousarray(cos_rep[bs]))
        sin_shards.append(np.ascontiguousarray(sin_rep[bs]))

    in_maps = []
    for core in range(N_CORES):
        hg, bg = core // BGQ, core % BGQ
        in_maps.append({
            "qdnT_dr": qT_shards[bg],
            "wq_nope": wqn_shards[hg],
            "wq_pe": wqpe_shards[hg],
            "w_ukvT": wukv_shards[hg],
            "w_uvT": wuv_shards[hg],
            "ckvT_sc": ckvT_shards[bg],
            "ckv_nc": ckvnc_shards[bg],
            "kpeT": kpeT_shards[bg],
            "ckvnewT": cknT_shards[bg],
            "ckvnew_nc": cknN_shards[bg],
            "knewT": knew_shards[bg],
            "cos_rep": cos_shards[bg],
            "sin_rep": sin_shards[bg],
        })
    return in_maps, (n_cached, B, H, bsz)


def assemble(results, meta):
    n_cached, B, H, bsz = meta
    out_full = np.empty((bsz, NUM_HEADS, V_HEAD), dtype=np.float32)
    for core in range(N_CORES):
        hg, bg = core // BGQ, core % BGQ
        out_full[bg * B:(bg + 1) * B, hg * H:(hg + 1) * H, :] = results[core]["out"]
    return out_full


def kernel(**inputs):
    in_maps, meta = prepare_in_maps(**inputs)
    n_cached, B, H, bsz = meta
    nc = _get_build(n_cached, B, H)
    res = run_bass_kernel_spmd(nc, in_maps, core_ids=list(range(N_CORES)))
    return assemble(res.results, meta)
